# revision 26
# baseline (speedup 1.0000x reference)
"""AdaptiveGNN (GCN+GAT+SAGE mixture) on 8 Trainium2 NeuronCores.

Strategy: destination-sharded graph parallelism, SINGLE NEFF launch.
 - Nodes split into 8 contiguous shards (6250 each, padded to 6272). Core k
   computes every per-node output row for shard k.
 - Edges (with self-loops where the op needs them) are sorted by destination
   on the host and padded into a static per-window tile schedule shared by
   all 8 cores (window = 128 destination rows -> one PSUM accumulation).
 - Halo exchange is done ON DEVICE: each core writes its shard of the node
   feature table into a DRAM bounce buffer, an AllGather builds the full
   table on every core, and per-edge indirect-DMA gathers read source rows
   from it. The host only ships the 1/8 x-shard + edge schedule per core
   (~3 MB/core instead of ~60 MB/core for host-side halo routing).
 - Per edge-tile: indirect gather of source rows, a one-hot "selection"
   matrix built from window-local destination ids (weighted by the per-edge
   coefficient: GCN norm / SAGE 1/deg / GAT exp(logit)), and a TensorE
   matmul that performs the segment-sum into PSUM.
 - Program phases (one launch): (1) per-window x processing -> x table
   [x|1|a_src] + a_dst table + column sums; (2) AllGather x/a_dst tables,
   AllReduce column sums, gate MLP; (3) layer 1 of all three branches
   (edge loop + window tails) -> layer-2 table [h1|h2|1|hs|a2src] + a2dst
   table; (4) AllGather layer-2 tables; (5) layer 2 of all three branches
   + gated mix -> final output rows.
"""

import sys

sys.path.insert(0, "/opt/trn_rl_repo")

import numpy as np

from concourse import bacc, bass, mybir, tile
from concourse.bass_utils import run_bass_kernel_spmd
import concourse.tile_sem_assignment as _tsa

# Clamp Tile's DMA-completion semaphore lanes. The kernel-tail Drain waits on
# every producer semaphore, and walrus' codegen rejects instructions with too
# many sync waits; fewer lanes keeps the wait list within the ISA limit.
_tsa.NUM_HWDGE_SEMS = 8
_tsa.NUM_SWDGE_GLOBAL_SEMS = 8

F32 = mybir.dt.float32
F16 = mybir.dt.float16
I32 = mybir.dt.int32
U8 = mybir.dt.uint8
U16 = mybir.dt.uint16
AF = mybir.ActivationFunctionType
ALU = mybir.AluOpType

NC_N = 8          # cores
D = 64            # feature dim
H1 = 4            # GAT hidden heads
NEG_SLOPE = 0.2
BN_EPS = 1e-5
CW1 = D + 1 + H1 + 1      # x-table row: [x | 1 | a_src | dis]       (70)
CW2 = 3 * D + 1 + 1 + 1   # l2-table row: [h1 | h2 | 1 | hs | a2src | dis] (195)
ADW = H1 + 1              # a_dst-table row: [a_dst | dis]            (5)
A2W = 2                   # layer-2 dst-table row: [a2dst | dis]

# weight-blob layout (host packs, device slices) — order matters
WSPEC = [
    ("vcat", (D, 2 * H1)),
    ("gw1", (D, D)), ("gb1", (1, D)), ("gw2", (D, 3)), ("gb2", (1, 3)),
    ("gcn_w1", (D, D)), ("gcn1_s", (D, 1)), ("gcn1_b", (D, 1)),
    ("sage_wl1", (D, D)), ("sage_wr1", (D, D)), ("sage_bl1", (D, 1)),
    ("w2A", (128, D)), ("w2B", (128, D)), ("v2u2", (128, 4)),
    ("w1h", (D, 4 * D)), ("b1c", (128, 2)),
    ("gcn_w2", (D, D)), ("gcn_b2c", (D, 1)),
    ("sage_wl2", (D, D)), ("sage_wr2", (D, D)),
    ("sage_bl2c", (D, 1)), ("gat_b2r", (1, D)),
]
WTOT = sum(r * c for _, (r, c) in WSPEC)
WSH = ((WTOT + NC_N * 64 - 1) // (NC_N * 64)) * 64   # weight-blob shard


# ----------------------------------------------------------------- host prep
def build_schedule(edge_index, n_nodes):
    """Sort edges (plus self-loops) by destination, shard by destination,
    and produce a tile schedule common to all cores plus per-core streams.
    Source/destination node ids are remapped to AllGather-table row space:
    node n -> (n // shard) * npad + (n % shard)."""
    shard = n_nodes // NC_N
    nw = (shard + 127) // 128
    npad = nw * 128
    row = edge_index[0].astype(np.int64)
    col = edge_index[1].astype(np.int64)
    loops = np.arange(n_nodes, dtype=np.int64)
    r_all = np.concatenate([row, loops])
    c_all = np.concatenate([col, loops])

    # GCN symmetric normalization (self-loops included)
    deg = np.bincount(c_all, minlength=n_nodes).astype(np.float64)
    dis = np.where(deg > 0, deg ** -0.5, 0.0)
    # SAGE mean weights (real edges only; appended self-loops get cnt=0,
    # which the device maps to weight 0)
    cnt = np.bincount(col, minlength=n_nodes)
    assert cnt.max() < 256, "dst in-degree must fit in u8"
    cnt_e_all = np.concatenate(
        [cnt[col], np.zeros(n_nodes, np.int64)]).astype(np.int64)
    # table-row remap
    tr_all = (r_all // shard) * npad + (r_all % shard)

    per_core = []
    counts = np.zeros((NC_N, nw), dtype=np.int64)
    for k in range(NC_N):
        lo, hi = k * shard, (k + 1) * shard
        sel = np.nonzero((c_all >= lo) & (c_all < hi))[0]
        cl = c_all[sel] - lo
        order = np.argsort(cl, kind="stable")
        sel = sel[order]
        cl = cl[order]
        w_of = cl // 128
        cnts = np.bincount(w_of, minlength=nw)
        counts[k] = cnts
        per_core.append((sel, cl, cnts))

    tiles_w = np.maximum(1, (counts.max(axis=0) + 127) // 128)
    Tpad = int(tiles_w.sum())

    streams = []
    for k in range(NC_N):
        sel, cl, cnts = per_core[k]
        idx_row = np.zeros(Tpad * 128, np.int32)
        # packed u16: low 8 bits colrel (255 pad), high 8 bits dst in-degree
        ccomb = np.full(Tpad * 128, 255, np.uint16)
        pos = 0      # position in padded stream
        epos = 0     # position in this core's sorted edge list
        for w in range(nw):
            cw = int(cnts[w])
            seg = sel[epos:epos + cw]
            base = pos
            idx_row[base:base + cw] = tr_all[seg]
            ccomb[base:base + cw] = (
                (cl[epos:epos + cw] % 128) + 256 * cnt_e_all[seg]
            ).astype(np.uint16)
            epos += cw
            pos += int(tiles_w[w]) * 128
        kb = np.full((128, 1), k * npad, np.uint16)
        dis_pad = np.zeros(npad, np.float16)
        dis_pad[:shard] = dis[k * shard:(k + 1) * shard]
        st = {
            # [ idx_row | colrel+cnt | k*npad ] as uint16 (all < 65536)
            "iu16": np.concatenate(
                [idx_row.reshape(Tpad, 128).T.astype(np.uint16),
                 ccomb.reshape(Tpad, 128).T,
                 kb], axis=1).copy(),
            "dis16": dis_pad,
        }
        streams.append(st)
    return streams, [int(t) for t in tiles_w], Tpad, shard, nw


# ------------------------------------------------------------- common pieces
def _load_w(nc, pool, dram, shape, tag):
    ld = pool.tile(list(shape), F32, tag=tag + "_ld")
    nc.sync.dma_start(out=ld[:], in_=dram[:])
    t = pool.tile(list(shape), F32, tag=tag)
    nc.vector.tensor_copy(t[:], ld[:])
    return t


def _stage_out_dma(nc, st_tile, dram, nw, width):
    # staging [128, nw*width] -> dram [nw*128, width]
    out_ap = bass.AP(dram, 0, [[width, 128], [128 * width, nw], [1, width]])
    nc.sync.dma_start(out=out_ap, in_=st_tile[:].rearrange("p (w c) -> p w c", w=nw))


# ----------------------------------------------------------- the one program
def build_all(n_nodes, shard, nw, tiles_w, Tpad):
    npad = nw * 128
    ntot = NC_N * npad
    rg = [list(range(NC_N))]
    nc = bacc.Bacc(num_devices=NC_N)
    # flat f16 blob: [ xs (npad*D) | dis (npad) | weight shard (WSH) ]
    FB_DIS = npad * D
    FB_WB = FB_DIS + npad
    N16 = FB_WB + WSH
    dr = {
        "fb": nc.dram_tensor("fb", [1, N16], F16, kind="ExternalInput"),
        # u16 pack: [ idx_row (Tpad) | colrel (Tpad) | k*npad (1) ]
        "iu16": nc.dram_tensor("iu16", [128, 2 * Tpad + 1], U16,
                               kind="ExternalInput"),
    }
    out = nc.dram_tensor("out", [npad, D], F16, kind="ExternalOutput")

    def fb_xs_ap(w):
        # window w of the x shard: rows w*128..w*128+127, D cols
        return bass.AP(dr["fb"], w * 128 * D, [[D, 128], [1, D]])
    cident = nc.inline_tensor(np.eye(128, dtype=np.float32), name="cident")
    ciota = nc.inline_tensor(
        np.tile(np.arange(128, dtype=np.float32), (128, 1)), name="ciota")

    with tile.TileContext(nc) as tc:
        with (
            tc.tile_pool(name="const", bufs=1) as const,
            tc.tile_pool(name="wts", bufs=1) as wts,
            tc.tile_pool(name="stream", bufs=1) as stream,
            tc.tile_pool(name="stage", bufs=1) as stage,
            tc.tile_pool(name="gat", bufs=8) as gat,
            tc.tile_pool(name="m", bufs=8) as mpool,
            tc.tile_pool(name="sm", bufs=3) as sm,
            tc.tile_pool(name="tl", bufs=4) as tl,
            tc.tile_pool(name="dram", bufs=1, space="DRAM") as dram,
            tc.tile_pool(name="pacc", bufs=1, space="PSUM") as pacc,
            tc.tile_pool(name="ptmp", bufs=2, space="PSUM") as ptmp,
        ):
            # ---- constants
            ident = _load_w(nc, const, cident, (128, 128), "ident")
            iota_f = _load_w(nc, const, ciota, (128, 128), "iota_f")
            ones_col = const.tile([128, 1], F32, tag="ones_col")
            nc.vector.memset(ones_col[:], 1.0)
            ones_row = const.tile([1, 128], F32, tag="ones_row")
            nc.vector.memset(ones_row[:], 1.0)

            # ---- weights: AllGather the 1/8 blob shards, then slice to SBUF
            wb_in = dram.tile([1, WSH], F16, tag="wb_in")
            wbfull = dram.tile([1, NC_N * WSH], F16, tag="wbfull")
            nc.gpsimd.dma_start(
                wb_in[:], bass.AP(dr["fb"], FB_WB, [[1, 1], [1, WSH]]))
            nc.gpsimd.collective_compute(
                "AllGather", ALU.bypass, replica_groups=rg,
                ins=[wb_in.opt()], outs=[wbfull.opt()])
            W = {}
            woff = 0
            for nm, (r, c) in WSPEC:
                ld = wts.tile([r, c], F16, tag=nm + "_ld")
                nc.sync.dma_start(
                    out=ld[:],
                    in_=bass.AP(wbfull[:].tensor, woff, [[c, r], [1, c]]))
                t = wts.tile([r, c], F32, tag=nm)
                nc.vector.tensor_copy(t[:], ld[:])
                W[nm] = t
                woff += r * c

            # ---- edge streams to SBUF (unpack + upconvert)
            iu = stream.tile([128, 2 * Tpad + 1], U16, tag="iu")
            nc.sync.dma_start(out=iu[:], in_=dr["iu16"][:])
            idxr = stream.tile([128, Tpad], I32, tag="idxr")
            nc.vector.tensor_copy(idxr[:], iu[:, 0:Tpad])
            # packed column: low 8 bits = colrel (255 pad), high 8 = dst cnt
            cc = stream.tile([128, Tpad], I32, tag="cc")
            nc.vector.tensor_copy(cc[:], iu[:, Tpad:2 * Tpad])
            cri = stream.tile([128, Tpad], I32, tag="cri")
            nc.vector.tensor_scalar(out=cri[:], in0=cc[:], scalar1=255,
                                    scalar2=None, op0=ALU.bitwise_and)
            crf = stream.tile([128, Tpad], F32, tag="crf")
            nc.vector.tensor_copy(crf[:], cri[:])
            cnti = stream.tile([128, Tpad], I32, tag="cnti")
            nc.vector.tensor_scalar(out=cnti[:], in0=cc[:], scalar1=8,
                                    scalar2=None, op0=ALU.logical_shift_right)
            cntf = stream.tile([128, Tpad], F32, tag="cntf")
            nc.vector.tensor_copy(cntf[:], cnti[:])
            # wsage = (cnt > 0) ? 1/cnt : 0   (cnt==0 marks appended loops)
            c1s = stream.tile([128, Tpad], F32, tag="c1s")
            nc.vector.tensor_scalar(out=c1s[:], in0=cntf[:], scalar1=1.0,
                                    scalar2=None, op0=ALU.max)
            rcs = stream.tile([128, Tpad], F32, tag="rcs")
            nc.vector.reciprocal(rcs[:], c1s[:])
            mzs = stream.tile([128, Tpad], F32, tag="mzs")
            nc.vector.tensor_scalar(out=mzs[:], in0=cntf[:], scalar1=0.0,
                                    scalar2=None, op0=ALU.is_gt)
            wsg = stream.tile([128, Tpad], F32, tag="wsg")
            nc.vector.tensor_tensor(out=wsg[:], in0=rcs[:], in1=mzs[:],
                                    op=ALU.mult)
            kbf = stream.tile([128, 1], F32, tag="kbf")
            nc.vector.tensor_copy(kbf[:], iu[:, 2 * Tpad:2 * Tpad + 1])
            # derive the dst-row gather stream on device:
            #   idx_dst[p, t] = k*npad + win(t)*128 + min(colrel[p, t], 127)
            # (clamp keeps padded entries in-bounds; their M columns are 0)
            idxd_f = stream.tile([128, Tpad], F32, tag="idxd_f")
            nc.vector.tensor_scalar(out=idxd_f[:], in0=crf[:], scalar1=127.0,
                                    scalar2=kbf[:, :1], op0=ALU.min,
                                    op1=ALU.add)
            tg = 0
            for w in range(nw):
                for _ in range(tiles_w[w]):
                    if w:
                        nc.vector.tensor_scalar(
                            out=idxd_f[:, tg:tg + 1], in0=idxd_f[:, tg:tg + 1],
                            scalar1=float(w * 128), scalar2=None, op0=ALU.add)
                    tg += 1
            idxd = stream.tile([128, Tpad], I32, tag="idxd")
            nc.vector.tensor_copy(idxd[:], idxd_f[:])
            # column accessors into the packed stream tiles
            s_idx_row = lambda g: idxr[:, g:g + 1]
            s_idx_dst = lambda g: idxd[:, g:g + 1]
            s_colrel = lambda t: crf[:, t:t + 1]
            s_wsage = lambda t: wsg[:, t:t + 1]

            # ---- DRAM bounce buffers (collective in/out)
            xtab_in = dram.tile([npad, CW1], F32, tag="xtab_in")
            xtab = dram.tile([ntot, CW1], F32, tag="xtab")
            adtab_in = dram.tile([npad, ADW], F32, tag="adtab_in")
            adtab = dram.tile([ntot, ADW], F32, tag="adtab")
            cs_in = dram.tile([D, 1], F32, tag="cs_in")
            cs_out = dram.tile([D, 1], F32, tag="cs_out")
            tab2_in = dram.tile([npad, CW2], F32, tag="tab2_in")
            tab2 = dram.tile([ntot, CW2], F32, tag="tab2")
            a2tab_in = dram.tile([npad, A2W], F32, tag="a2tab_in")
            a2tab = dram.tile([ntot, A2W], F32, tag="a2tab")

            # ---- SBUF staging that lives across phases
            st_hs = stage.tile([128, nw * D], F32, tag="st_hs")
            st_out = stage.tile([128, nw * D], F16, tag="st_out")

            # ================= phase 1: per-window x processing =============
            csacc = stage.tile([D, 1], F32, tag="csacc")
            nc.vector.memset(csacc[:], 0.0)
            for w in range(nw):
                xt0 = tl.tile([128, D], F16, tag="xt0")
                nc.sync.dma_start(out=xt0[:], in_=fb_xs_ap(w))
                xt = tl.tile([128, D], F32, tag="xt")
                nc.vector.tensor_copy(xt[:], xt0[:])
                pT = ptmp.tile([D, 128], F32, tag="pt")
                nc.tensor.matmul(out=pT[:], lhsT=xt[:], rhs=ident[:],
                                 is_transpose=True)
                xT = tl.tile([D, 128], F32, tag="xT")
                nc.vector.tensor_copy(xT[:], pT[:])
                pa = ptmp.tile([2 * H1, 128], F32, tag="pt")
                nc.tensor.matmul(out=pa[:], lhsT=W["vcat"][:], rhs=xT[:])
                aT = tl.tile([2 * H1, 128], F32, tag="aT")
                nc.vector.tensor_copy(aT[:], pa[:])
                pb = ptmp.tile([128, 2 * H1], F32, tag="pt")
                nc.tensor.matmul(out=pb[:], lhsT=aT[:],
                                 rhs=ident[:2 * H1, :2 * H1],
                                 is_transpose=True)
                ab = tl.tile([128, 2 * H1], F32, tag="ab")
                nc.vector.tensor_copy(ab[:], pb[:])
                dis16 = tl.tile([128, 1], F16, tag="dis16")
                nc.sync.dma_start(
                    out=dis16[:],
                    in_=bass.AP(dr["fb"], FB_DIS + w * 128, [[1, 128], [1, 1]]))
                disw = tl.tile([128, 1], F32, tag="disw")
                nc.vector.tensor_copy(disw[:], dis16[:])
                xrow = tl.tile([128, CW1], F32, tag="xrow")
                nc.vector.tensor_copy(xrow[:, 0:D], xt[:])
                nc.vector.memset(xrow[:, D:D + 1], 1.0)
                nc.vector.tensor_copy(xrow[:, D + 1:D + 1 + H1], ab[:, 0:H1])
                nc.vector.tensor_copy(xrow[:, CW1 - 1:CW1], disw[:])
                nc.sync.dma_start(
                    out=xtab_in[w * 128:(w + 1) * 128, :], in_=xrow[:])
                adrow = tl.tile([128, ADW], F32, tag="adrow")
                nc.vector.tensor_copy(adrow[:, 0:H1], ab[:, H1:2 * H1])
                nc.vector.tensor_copy(adrow[:, H1:ADW], disw[:])
                nc.sync.dma_start(
                    out=adtab_in[w * 128:(w + 1) * 128, :], in_=adrow[:])
                csw = tl.tile([D, 1], F32, tag="csw")
                nc.vector.tensor_reduce(out=csw[:], in_=xT[:],
                                        axis=mybir.AxisListType.X, op=ALU.add)
                nc.vector.tensor_tensor(out=csacc[:], in0=csacc[:],
                                        in1=csw[:], op=ALU.add)
            nc.sync.dma_start(out=cs_in[:], in_=csacc[:])

            # ================= phase 2: collectives + gate MLP ==============
            nc.gpsimd.collective_compute(
                "AllGather", ALU.bypass, replica_groups=rg,
                ins=[xtab_in.opt()], outs=[xtab.opt()])
            nc.gpsimd.collective_compute(
                "AllGather", ALU.bypass, replica_groups=rg,
                ins=[adtab_in.opt()], outs=[adtab.opt()])
            nc.gpsimd.collective_compute(
                "AllReduce", ALU.add, replica_groups=rg,
                ins=[cs_in.opt()], outs=[cs_out.opt()])

            csg0 = sm.tile([D, 1], F32, tag="csg0")
            nc.sync.dma_start(out=csg0[:], in_=cs_out[:])
            xbT = sm.tile([D, 1], F32, tag="g_xbT")
            nc.vector.tensor_scalar(out=xbT[:], in0=csg0[:],
                                    scalar1=1.0 / n_nodes, scalar2=None,
                                    op0=ALU.mult)
            pg1 = ptmp.tile([1, D], F32, tag="pt")
            nc.tensor.matmul(out=pg1[:], lhsT=xbT[:], rhs=W["gw1"][:])
            g1 = sm.tile([1, D], F32, tag="g_g1")
            nc.vector.tensor_tensor(out=g1[:], in0=pg1[:], in1=W["gb1"][:],
                                    op=ALU.add)
            g1r = sm.tile([1, D], F32, tag="g_g1r")
            nc.vector.tensor_scalar(out=g1r[:], in0=g1[:], scalar1=0.0,
                                    scalar2=None, op0=ALU.max)
            pg1T = ptmp.tile([D, 1], F32, tag="pt")
            nc.tensor.matmul(out=pg1T[:], lhsT=g1r[:], rhs=ident[:1, :1],
                             is_transpose=True)
            g1T = sm.tile([D, 1], F32, tag="g_g1T")
            nc.vector.tensor_copy(g1T[:], pg1T[:])
            pg2 = ptmp.tile([1, 3], F32, tag="pt")
            nc.tensor.matmul(out=pg2[:], lhsT=g1T[:], rhs=W["gw2"][:])
            g2 = sm.tile([1, 3], F32, tag="g_g2")
            nc.vector.tensor_tensor(out=g2[:], in0=pg2[:], in1=W["gb2"][:],
                                    op=ALU.add)
            g2e = sm.tile([1, 3], F32, tag="g_g2e")
            nc.scalar.activation(out=g2e[:], in_=g2[:], func=AF.Exp)
            g2s = sm.tile([1, 1], F32, tag="g_g2s")
            nc.vector.tensor_reduce(out=g2s[:], in_=g2e[:],
                                    axis=mybir.AxisListType.X, op=ALU.add)
            g2r = sm.tile([1, 1], F32, tag="g_g2r")
            nc.vector.reciprocal(g2r[:], g2s[:])
            gate_sb = sm.tile([1, 3], F32, tag="g_gate")
            nc.vector.tensor_scalar(out=gate_sb[:], in0=g2e[:],
                                    scalar1=g2r[:, :1], scalar2=None,
                                    op0=ALU.mult)
            # gate scalar broadcasts
            pw128 = ptmp.tile([128, 3], F32, tag="pt")
            nc.tensor.matmul(out=pw128[:], lhsT=ones_row[:], rhs=gate_sb[:])
            wc = wts.tile([128, 3], F32, tag="wc")
            nc.vector.tensor_copy(wc[:], pw128[:])
            pw64 = ptmp.tile([D, 3], F32, tag="pt")
            nc.tensor.matmul(out=pw64[:], lhsT=ones_row[:1, :D],
                             rhs=gate_sb[:])
            w64 = wts.tile([D, 3], F32, tag="w64")
            nc.vector.tensor_copy(w64[:], pw64[:])
            b2w0 = wts.tile([D, 1], F32, tag="b2w0")
            nc.vector.tensor_scalar(out=b2w0[:], in0=W["gcn_b2c"][:],
                                    scalar1=w64[:, 0:1], scalar2=None,
                                    op0=ALU.mult)
            pbg = ptmp.tile([128, D], F32, tag="pt")
            nc.tensor.matmul(out=pbg[:], lhsT=ones_row[:], rhs=W["gat_b2r"][:])
            bgat = wts.tile([128, D], F32, tag="bgat")
            nc.vector.tensor_scalar(out=bgat[:], in0=pbg[:],
                                    scalar1=wc[:, 1:2], scalar2=None,
                                    op0=ALU.mult)

            # ================= phase 3: layer-1 edge loop ===================
            Gs, Es, Wn1 = [None] * Tpad, [None] * Tpad, [None] * Tpad

            def ensure_group1(g):
                if Gs[g] is not None:
                    return
                Gt = gat.tile([128, CW1], F32, tag="G")
                nc.gpsimd.indirect_dma_start(
                    out=Gt[:], out_offset=None, in_=xtab[:],
                    in_offset=bass.IndirectOffsetOnAxis(
                        ap=s_idx_row(g), axis=0))
                Gc = gat.tile([128, CW1], F32, tag="Gc")
                nc.vector.tensor_copy(Gc[:], Gt[:])
                At = gat.tile([128, ADW], F32, tag="At")
                nc.gpsimd.indirect_dma_start(
                    out=At[:], out_offset=None, in_=adtab[:],
                    in_offset=bass.IndirectOffsetOnAxis(
                        ap=s_idx_dst(g), axis=0))
                wn1 = gat.tile([128, 1], F32, tag="wn1")
                nc.vector.tensor_tensor(
                    out=wn1[:], in0=Gc[:, CW1 - 1:CW1], in1=At[:, H1:ADW],
                    op=ALU.mult)
                zt = gat.tile([128, H1], F32, tag="z")
                nc.vector.tensor_tensor(
                    out=zt[:], in0=Gc[:, D + 1:D + 1 + H1], in1=At[:, 0:H1],
                    op=ALU.add)
                zs = gat.tile([128, H1], F32, tag="zs")
                nc.vector.tensor_scalar(out=zs[:], in0=zt[:],
                                        scalar1=NEG_SLOPE, scalar2=None,
                                        op0=ALU.mult)
                nc.vector.tensor_tensor(out=zt[:], in0=zt[:], in1=zs[:],
                                        op=ALU.max)
                et = gat.tile([128, H1], F32, tag="E")
                nc.scalar.activation(out=et[:], in_=zt[:], func=AF.Exp)
                Gs[g], Es[g], Wn1[g] = Gc, et, wn1

            t_glob = 0
            for w in range(nw):
                ntw = tiles_w[w]
                p_gcnT = pacc.tile([D, 128], F32, tag="p_gcnT")
                p_sageT = pacc.tile([D, 128], F32, tag="p_sageT")
                p_gath = []
                for h in range(H1):
                    pg = pacc.tile([128, D + 1], F32, tag=f"p_gat{h}")
                    p_gath.append(pg)
                for t in range(ntw):
                    g = t_glob
                    ensure_group1(g)
                    Gc, et = Gs[g], Es[g]
                    g64 = Gc[:, 0:D]
                    g65 = Gc[:, 0:D + 1]
                    cr = s_colrel(t_glob)
                    st, sp = (t == 0), (t == ntw - 1)
                    Mg = mpool.tile([128, 128], F32, tag="Mg")
                    nc.vector.tensor_scalar(
                        out=Mg[:], in0=iota_f[:], scalar1=cr,
                        scalar2=Wn1[g][:, 0:1],
                        op0=ALU.is_equal, op1=ALU.mult)
                    nc.tensor.matmul(out=p_gcnT[:], lhsT=g64, rhs=Mg[:],
                                     start=st, stop=sp)
                    Ms = mpool.tile([128, 128], F32, tag="Ms")
                    nc.vector.tensor_scalar(
                        out=Ms[:], in0=iota_f[:], scalar1=cr,
                        scalar2=s_wsage(t_glob),
                        op0=ALU.is_equal, op1=ALU.mult)
                    nc.tensor.matmul(out=p_sageT[:], lhsT=g64, rhs=Ms[:],
                                     start=st, stop=sp)
                    for h in range(H1):
                        Mh = mpool.tile([128, 128], F32, tag="Mh")
                        nc.vector.tensor_scalar(
                            out=Mh[:], in0=iota_f[:], scalar1=cr,
                            scalar2=et[:, h:h + 1],
                            op0=ALU.is_equal, op1=ALU.mult)
                        nc.tensor.matmul(
                            out=p_gath[h][:], lhsT=Mh[:], rhs=g65,
                            start=st, stop=sp)
                    t_glob += 1

                # ---------- window tails ----------
                t2 = tl.tile([128, CW2], F32, tag="t2")
                nc.vector.memset(t2[:, 2 * D:2 * D + 1], 1.0)

                # GCN1: h1 = relu(s*(W1^T aggT) + b) -> t2[:, 0:D]
                aggT = tl.tile([D, 128], F32, tag="aggT")
                nc.vector.tensor_copy(aggT[:], p_gcnT[:])
                ph1T = ptmp.tile([D, 128], F32, tag="pt")
                nc.tensor.matmul(out=ph1T[:], lhsT=W["gcn_w1"][:], rhs=aggT[:])
                h1Ts = tl.tile([D, 128], F32, tag="h1Ts")
                nc.scalar.activation(out=h1Ts[:], in_=ph1T[:], func=AF.Relu,
                                     scale=W["gcn1_s"][:, :1],
                                     bias=W["gcn1_b"][:, :1])
                h1Tv = tl.tile([D, 128], F32, tag="h1Tv")
                nc.vector.tensor_copy(h1Tv[:], h1Ts[:])
                ph1 = ptmp.tile([128, D], F32, tag="pt")
                nc.tensor.matmul(out=ph1[:], lhsT=h1Tv[:], rhs=ident[:D, :D],
                                 is_transpose=True)
                nc.vector.tensor_copy(t2[:, 0:D], ph1[:])

                # GAT1 heads: head_h = (sum exp*x)/den ; x2T_h = W_h^T head_h^T
                x2TA = tl.tile([128, 128], F32, tag="x2TA")
                x2TB = tl.tile([128, 128], F32, tag="x2TB")
                for h in range(H1):
                    rd = tl.tile([128, 1], F32, tag="rd")
                    nc.vector.reciprocal(rd[:], p_gath[h][:, D:D + 1])
                    hd_sb = tl.tile([128, D], F32, tag="hd_sb")
                    nc.vector.tensor_scalar(
                        out=hd_sb[:], in0=p_gath[h][:, 0:D],
                        scalar1=rd[:, :1], scalar2=None, op0=ALU.mult)
                    pht = ptmp.tile([D, 128], F32, tag="pt")
                    nc.tensor.matmul(out=pht[:], lhsT=hd_sb[:], rhs=ident[:],
                                     is_transpose=True)
                    hdT = tl.tile([D, 128], F32, tag="hdT_g")
                    nc.vector.tensor_copy(hdT[:], pht[:])
                    pxh = ptmp.tile([D, 128], F32, tag="pt")
                    nc.tensor.matmul(out=pxh[:],
                                     lhsT=W["w1h"][:, h * D:(h + 1) * D],
                                     rhs=hdT[:])
                    stgt = x2TA if h < 2 else x2TB
                    nc.vector.tensor_copy(
                        stgt[(h % 2) * D:(h % 2 + 1) * D, :], pxh[:])
                x2T = []
                for half, px in enumerate((x2TA, x2TB)):
                    yT = tl.tile([128, 128], F32, tag="yT")
                    nc.vector.tensor_scalar(
                        out=yT[:], in0=px[:],
                        scalar1=W["b1c"][:, half:half + 1], scalar2=None,
                        op0=ALU.add)
                    ymin = tl.tile([128, 128], F32, tag="ymin")
                    nc.vector.tensor_scalar(out=ymin[:], in0=yT[:],
                                            scalar1=0.0, scalar2=None,
                                            op0=ALU.min)
                    yexp = tl.tile([128, 128], F32, tag="yexp")
                    nc.scalar.activation(out=yexp[:], in_=ymin[:], func=AF.Exp)
                    ye1 = tl.tile([128, 128], F32, tag="ye1")
                    nc.vector.tensor_scalar(out=ye1[:], in0=yexp[:],
                                            scalar1=-1.0, scalar2=None,
                                            op0=ALU.add)
                    ymax = tl.tile([128, 128], F32, tag="ymax")
                    nc.vector.tensor_scalar(out=ymax[:], in0=yT[:],
                                            scalar1=0.0, scalar2=None,
                                            op0=ALU.max)
                    xt2 = tl.tile([128, 128], F32, tag=f"x2T{half}")
                    nc.vector.tensor_tensor(out=xt2[:], in0=ymax[:],
                                            in1=ye1[:], op=ALU.add)
                    x2T.append(xt2)
                ph2T = ptmp.tile([D, 128], F32, tag="pt")
                nc.tensor.matmul(out=ph2T[:], lhsT=W["w2A"][:], rhs=x2T[0][:],
                                 start=True, stop=False)
                nc.tensor.matmul(out=ph2T[:], lhsT=W["w2B"][:], rhs=x2T[1][:],
                                 start=False, stop=True)
                pa2T = ptmp.tile([2, 128], F32, tag="pt")
                nc.tensor.matmul(out=pa2T[:], lhsT=W["v2u2"][:, 0:2],
                                 rhs=x2T[0][:], start=True, stop=False)
                nc.tensor.matmul(out=pa2T[:], lhsT=W["v2u2"][:, 2:4],
                                 rhs=x2T[1][:], start=False, stop=True)
                h2Ts = tl.tile([D, 128], F32, tag="h2Ts")
                nc.vector.tensor_copy(h2Ts[:], ph2T[:])
                ph2 = ptmp.tile([128, D], F32, tag="pt")
                nc.tensor.matmul(out=ph2[:], lhsT=h2Ts[:], rhs=ident[:D, :D],
                                 is_transpose=True)
                nc.vector.tensor_copy(t2[:, D:2 * D], ph2[:])
                a2Ts = tl.tile([2, 128], F32, tag="a2Ts")
                nc.vector.tensor_copy(a2Ts[:], pa2T[:])
                pa2 = ptmp.tile([128, 2], F32, tag="pt")
                nc.tensor.matmul(out=pa2[:], lhsT=a2Ts[:], rhs=ident[:2, :2],
                                 is_transpose=True)
                nc.vector.tensor_copy(t2[:, CW2 - 2:CW2 - 1], pa2[:, 0:1])
                dis16b = tl.tile([128, 1], F16, tag="dis16b")
                nc.sync.dma_start(
                    out=dis16b[:],
                    in_=bass.AP(dr["fb"], FB_DIS + w * 128, [[1, 128], [1, 1]]))
                diswb = tl.tile([128, 1], F32, tag="diswb")
                nc.vector.tensor_copy(diswb[:], dis16b[:])
                nc.vector.tensor_copy(t2[:, CW2 - 1:CW2], diswb[:])
                a2row = tl.tile([128, A2W], F32, tag="a2row")
                nc.vector.tensor_copy(a2row[:, 0:1], pa2[:, 1:2])
                nc.vector.tensor_copy(a2row[:, 1:2], diswb[:])
                nc.sync.dma_start(
                    out=a2tab_in[w * 128:(w + 1) * 128, :], in_=a2row[:])

                # SAGE1 -> st_hs and t2[:, 2D+1:3D+1]
                meanT = tl.tile([D, 128], F32, tag="meanT")
                nc.vector.tensor_copy(meanT[:], p_sageT[:])
                xd0 = tl.tile([128, D], F16, tag="xd0")
                nc.sync.dma_start(out=xd0[:], in_=fb_xs_ap(w))
                xd = tl.tile([128, D], F32, tag="xd")
                nc.vector.tensor_copy(xd[:], xd0[:])
                pxdT = ptmp.tile([D, 128], F32, tag="pt")
                nc.tensor.matmul(out=pxdT[:], lhsT=xd[:], rhs=ident[:],
                                 is_transpose=True)
                xdT = tl.tile([D, 128], F32, tag="xdT")
                nc.vector.tensor_copy(xdT[:], pxdT[:])
                psT = ptmp.tile([D, 128], F32, tag="pt")
                nc.tensor.matmul(out=psT[:], lhsT=W["sage_wl1"][:],
                                 rhs=meanT[:], start=True, stop=False)
                nc.tensor.matmul(out=psT[:], lhsT=W["sage_wr1"][:],
                                 rhs=xdT[:], start=False, stop=True)
                sTs = tl.tile([D, 128], F32, tag="sTs")
                nc.scalar.activation(out=sTs[:], in_=psT[:], func=AF.Identity,
                                     bias=W["sage_bl1"][:, :1])
                sTv = tl.tile([D, 128], F32, tag="sTv")
                nc.vector.tensor_copy(sTv[:], sTs[:])
                ps_ = ptmp.tile([128, D], F32, tag="pt")
                nc.tensor.matmul(out=ps_[:], lhsT=sTv[:], rhs=ident[:D, :D],
                                 is_transpose=True)
                s_sb = tl.tile([128, D], F32, tag="s_sb")
                nc.vector.tensor_copy(s_sb[:], ps_[:])
                sq = tl.tile([128, D], F32, tag="sq")
                nc.vector.tensor_tensor(out=sq[:], in0=s_sb[:], in1=s_sb[:],
                                        op=ALU.mult)
                ssum = tl.tile([128, 1], F32, tag="ssum")
                nc.vector.tensor_reduce(out=ssum[:], in_=sq[:],
                                        axis=mybir.AxisListType.X, op=ALU.add)
                nc.vector.tensor_scalar(out=ssum[:], in0=ssum[:],
                                        scalar1=1e-24, scalar2=None,
                                        op0=ALU.add)
                rs = tl.tile([128, 1], F32, tag="rs")
                nc.vector.reciprocal(rs[:], ssum[:])
                rq = tl.tile([128, 1], F32, tag="rq")
                nc.scalar.activation(out=rq[:], in_=rs[:], func=AF.Sqrt)
                nc.vector.tensor_scalar(out=st_hs[:, w * D:(w + 1) * D],
                                        in0=s_sb[:], scalar1=rq[:, :1],
                                        scalar2=0.0, op0=ALU.mult,
                                        op1=ALU.max)
                nc.vector.tensor_copy(t2[:, 2 * D + 1:3 * D + 1],
                                      st_hs[:, w * D:(w + 1) * D])
                nc.sync.dma_start(
                    out=tab2_in[w * 128:(w + 1) * 128, :], in_=t2[:])

            # ================= phase 4: layer-2 AllGathers ==================
            nc.gpsimd.collective_compute(
                "AllGather", ALU.bypass, replica_groups=rg,
                ins=[tab2_in.opt()], outs=[tab2.opt()])
            nc.gpsimd.collective_compute(
                "AllGather", ALU.bypass, replica_groups=rg,
                ins=[a2tab_in.opt()], outs=[a2tab.opt()])

            # ================= phase 5: layer-2 edge loop ===================
            G2s, E2s, Wn2 = [None] * Tpad, [None] * Tpad, [None] * Tpad

            def ensure_group2(g):
                if G2s[g] is not None:
                    return
                G0 = gat.tile([128, CW2], F32, tag="G2")
                nc.gpsimd.indirect_dma_start(
                    out=G0[:], out_offset=None, in_=tab2[:],
                    in_offset=bass.IndirectOffsetOnAxis(
                        ap=s_idx_row(g), axis=0))
                Gc = gat.tile([128, CW2], F32, tag="G2c")
                nc.vector.tensor_copy(Gc[:], G0[:])
                A0 = gat.tile([128, A2W], F32, tag="A2t")
                nc.gpsimd.indirect_dma_start(
                    out=A0[:], out_offset=None, in_=a2tab[:],
                    in_offset=bass.IndirectOffsetOnAxis(
                        ap=s_idx_dst(g), axis=0))
                wn2 = gat.tile([128, 1], F32, tag="wn2")
                nc.vector.tensor_tensor(
                    out=wn2[:], in0=Gc[:, CW2 - 1:CW2], in1=A0[:, 1:2],
                    op=ALU.mult)
                z2 = gat.tile([128, 1], F32, tag="z2")
                nc.vector.tensor_tensor(
                    out=z2[:], in0=Gc[:, CW2 - 2:CW2 - 1], in1=A0[:, 0:1],
                    op=ALU.add)
                z2s = gat.tile([128, 1], F32, tag="z2s")
                nc.vector.tensor_scalar(out=z2s[:], in0=z2[:],
                                        scalar1=NEG_SLOPE, scalar2=None,
                                        op0=ALU.mult)
                nc.vector.tensor_tensor(out=z2[:], in0=z2[:], in1=z2s[:],
                                        op=ALU.max)
                e2 = gat.tile([128, 1], F32, tag="E2")
                nc.scalar.activation(out=e2[:], in_=z2[:], func=AF.Exp)
                G2s[g], E2s[g], Wn2[g] = Gc, e2, wn2

            t_glob = 0
            for w in range(nw):
                ntw = tiles_w[w]
                p_g2T = pacc.tile([D, 128], F32, tag="p_gcnT")
                p_s2T = pacc.tile([D, 128], F32, tag="p_sageT")
                p_gat2 = pacc.tile([128, D + 1], F32, tag="p_gat0")
                for t in range(ntw):
                    g = t_glob
                    ensure_group2(g)
                    Gc, e2 = G2s[g], E2s[g]
                    g1s = Gc[:, 0:D]
                    g2s_ = Gc[:, D:2 * D + 1]
                    g3s = Gc[:, 2 * D + 1:3 * D + 1]
                    cr = s_colrel(t_glob)
                    st, sp = (t == 0), (t == ntw - 1)
                    Mg = mpool.tile([128, 128], F32, tag="Mg")
                    nc.vector.tensor_scalar(
                        out=Mg[:], in0=iota_f[:], scalar1=cr,
                        scalar2=Wn2[g][:, 0:1],
                        op0=ALU.is_equal, op1=ALU.mult)
                    nc.tensor.matmul(out=p_g2T[:], lhsT=g1s, rhs=Mg[:],
                                     start=st, stop=sp)
                    Ms = mpool.tile([128, 128], F32, tag="Ms")
                    nc.vector.tensor_scalar(
                        out=Ms[:], in0=iota_f[:], scalar1=cr,
                        scalar2=s_wsage(t_glob),
                        op0=ALU.is_equal, op1=ALU.mult)
                    nc.tensor.matmul(out=p_s2T[:], lhsT=g3s, rhs=Ms[:],
                                     start=st, stop=sp)
                    Mh = mpool.tile([128, 128], F32, tag="Mh")
                    nc.vector.tensor_scalar(
                        out=Mh[:], in0=iota_f[:], scalar1=cr,
                        scalar2=e2[:, 0:1],
                        op0=ALU.is_equal, op1=ALU.mult)
                    nc.tensor.matmul(out=p_gat2[:], lhsT=Mh[:], rhs=g2s_,
                                     start=st, stop=sp)
                    t_glob += 1

                # GCN2 (+w0, +w0*b2)
                aggT = tl.tile([D, 128], F32, tag="aggT")
                nc.vector.tensor_copy(aggT[:], p_g2T[:])
                poT = ptmp.tile([D, 128], F32, tag="pt")
                nc.tensor.matmul(out=poT[:], lhsT=W["gcn_w2"][:], rhs=aggT[:])
                oTs = tl.tile([D, 128], F32, tag="oTs")
                nc.scalar.activation(out=oTs[:], in_=poT[:], func=AF.Identity,
                                     scale=w64[:, 0:1], bias=b2w0[:, :1])
                oTv = tl.tile([D, 128], F32, tag="oTv")
                nc.vector.tensor_copy(oTv[:], oTs[:])
                po = ptmp.tile([128, D], F32, tag="pt")
                nc.tensor.matmul(out=po[:], lhsT=oTv[:], rhs=ident[:D, :D],
                                 is_transpose=True)
                ogcn = tl.tile([128, D], F32, tag="ogcn")
                nc.vector.tensor_copy(ogcn[:], po[:])

                # GAT2 (+w1)
                rd = tl.tile([128, 1], F32, tag="rd")
                nc.vector.reciprocal(rd[:], p_gat2[:, D:D + 1])
                ogat = tl.tile([128, D], F32, tag="ogat")
                nc.vector.tensor_scalar(out=ogat[:], in0=p_gat2[:, 0:D],
                                        scalar1=rd[:, :1],
                                        scalar2=wc[:, 1:2],
                                        op0=ALU.mult, op1=ALU.mult)

                # SAGE2 (+w2); self input comes from st_hs staging
                meanT = tl.tile([D, 128], F32, tag="meanT")
                nc.vector.tensor_copy(meanT[:], p_s2T[:])
                phdT = ptmp.tile([D, 128], F32, tag="pt")
                nc.tensor.matmul(out=phdT[:],
                                 lhsT=st_hs[:, w * D:(w + 1) * D],
                                 rhs=ident[:], is_transpose=True)
                hdT = tl.tile([D, 128], F32, tag="hdT")
                nc.vector.tensor_copy(hdT[:], phdT[:])
                psT = ptmp.tile([D, 128], F32, tag="pt")
                nc.tensor.matmul(out=psT[:], lhsT=W["sage_wl2"][:],
                                 rhs=meanT[:], start=True, stop=False)
                nc.tensor.matmul(out=psT[:], lhsT=W["sage_wr2"][:],
                                 rhs=hdT[:], start=False, stop=True)
                sTs = tl.tile([D, 128], F32, tag="sTs")
                nc.scalar.activation(out=sTs[:], in_=psT[:], func=AF.Identity,
                                     bias=W["sage_bl2c"][:, :1])
                sTv = tl.tile([D, 128], F32, tag="sTv")
                nc.vector.tensor_copy(sTv[:], sTs[:])
                ps_ = ptmp.tile([128, D], F32, tag="pt")
                nc.tensor.matmul(out=ps_[:], lhsT=sTv[:], rhs=ident[:D, :D],
                                 is_transpose=True)
                s_sb = tl.tile([128, D], F32, tag="s_sb")
                nc.vector.tensor_copy(s_sb[:], ps_[:])
                sq = tl.tile([128, D], F32, tag="sq")
                nc.vector.tensor_tensor(out=sq[:], in0=s_sb[:], in1=s_sb[:],
                                        op=ALU.mult)
                ssum = tl.tile([128, 1], F32, tag="ssum")
                nc.vector.tensor_reduce(out=ssum[:], in_=sq[:],
                                        axis=mybir.AxisListType.X, op=ALU.add)
                nc.vector.tensor_scalar(out=ssum[:], in0=ssum[:],
                                        scalar1=1e-24, scalar2=None,
                                        op0=ALU.add)
                rs = tl.tile([128, 1], F32, tag="rs")
                nc.vector.reciprocal(rs[:], ssum[:])
                rq = tl.tile([128, 1], F32, tag="rq")
                nc.scalar.activation(out=rq[:], in_=rs[:], func=AF.Sqrt)
                osage = tl.tile([128, D], F32, tag="osage")
                nc.vector.tensor_scalar(out=osage[:], in0=s_sb[:],
                                        scalar1=rq[:, :1],
                                        scalar2=wc[:, 2:3],
                                        op0=ALU.mult, op1=ALU.mult)

                # mix
                mx1 = tl.tile([128, D], F32, tag="mx1")
                nc.vector.tensor_tensor(out=mx1[:], in0=ogcn[:], in1=ogat[:],
                                        op=ALU.add)
                mx2 = tl.tile([128, D], F32, tag="mx2")
                nc.vector.tensor_tensor(out=mx2[:], in0=mx1[:], in1=osage[:],
                                        op=ALU.add)
                nc.vector.tensor_tensor(out=st_out[:, w * D:(w + 1) * D],
                                        in0=mx2[:], in1=bgat[:], op=ALU.add)

            _stage_out_dma(nc, st_out, out, nw, D)
    return nc


# ---------------------------------------------------------------- host logic
DEBUG = {}
_PROG_CACHE = {}
_RUNNER_CACHE = {}


def _make_runner(nc):
    """Build a cached jit'd PJRT runner for a finalized Bass program.

    Mirrors run_bass_via_pjrt, but (a) the jit closure is built once and
    reused across calls (no per-call retrace / HLO rebuild), and (b) the
    donated output buffers are created sharded ON DEVICE (jnp.zeros with a
    NamedSharding) instead of being shipped from the host on every call.
    """
    import jax
    import jax.numpy as jnp
    from jax.experimental.shard_map import shard_map
    from jax.sharding import Mesh, PartitionSpec, NamedSharding
    from concourse import bass2jax
    bass2jax.install_neuronx_cc_hook()
    partition_name = (nc.partition_id_tensor.name
                      if nc.partition_id_tensor else None)
    in_names, out_names, out_avals = [], [], []
    for alloc in nc.m.functions[0].allocations:
        if not isinstance(alloc, mybir.MemoryLocationSet):
            continue
        name = alloc.memorylocations[0].name
        if alloc.kind == "ExternalInput":
            if name != partition_name:
                in_names.append(name)
        elif alloc.kind == "ExternalOutput":
            out_names.append(name)
            out_avals.append(jax.core.ShapedArray(
                tuple(alloc.tensor_shape), mybir.dt.np(alloc.dtype)))
    full_in_names = tuple(in_names + out_names +
                          ([partition_name] if partition_name else []))

    def _body(*args):
        operands = list(args)
        if partition_name is not None:
            operands.append(bass2jax.partition_id_tensor())
        outs = bass2jax._bass_exec_p.bind(
            *operands, out_avals=tuple(out_avals), in_names=full_in_names,
            out_names=tuple(out_names), lowering_input_output_aliases=(),
            sim_require_finite=True, sim_require_nnan=True, nc=nc)
        return tuple(outs)

    devices = jax.devices()[:NC_N]
    mesh = Mesh(np.asarray(devices), ("core",))
    sharding = NamedSharding(mesh, PartitionSpec("core"))
    n_p, n_o = len(in_names), len(out_names)
    fn = jax.jit(
        shard_map(_body, mesh=mesh,
                  in_specs=(PartitionSpec("core"),) * (n_p + n_o),
                  out_specs=(PartitionSpec("core"),) * n_o,
                  check_rep=False),
        keep_unused=True)

    # Persistent device-resident zero buffers for the output operands.
    # The NEFF writes every output element into the PJRT result buffers,
    # so these are never donated/consumed and can be reused across calls.
    zeros = [jnp.zeros((NC_N * a.shape[0], *a.shape[1:]), a.dtype,
                       device=sharding) for a in out_avals]
    jax.block_until_ready(zeros)

    def make_zeros():
        return zeros

    return fn, in_names, out_names, out_avals, make_zeros, devices, sharding


_EXPOOL = None


def _pool():
    global _EXPOOL
    if _EXPOOL is None:
        from concurrent.futures import ThreadPoolExecutor
        _EXPOOL = ThreadPoolExecutor(NC_N)
    return _EXPOOL


def _run(nc, in_maps):
    import time as _time
    import jax
    if not nc.is_finalized():
        nc.finalize()   # Bacc.compile(): reg alloc + sync-wait legalization
    key = id(nc)
    if key not in _RUNNER_CACHE:
        _RUNNER_CACHE[key] = _make_runner(nc)
    (fn, in_names, out_names, out_avals, make_zeros,
     devices, sharding) = _RUNNER_CACHE[key]
    ex = _pool()
    t0 = _time.perf_counter()

    # per-shard upload (parallel streams beat one big serialized transfer
    # over the axon tunnel)
    def put_arg(nm):
        futs = [jax.device_put(in_maps[k][nm], devices[k])
                for k in range(NC_N)]
        a0 = in_maps[0][nm]
        gshape = (NC_N * a0.shape[0],) + tuple(a0.shape[1:])
        return jax.make_array_from_single_device_arrays(
            gshape, sharding, futs)

    dev_args = list(ex.map(put_arg, in_names))
    outs = fn(*dev_args, *make_zeros())
    # per-shard download, in parallel
    fetched = []
    for i in range(len(out_names)):
        shards = sorted(outs[i].addressable_shards,
                        key=lambda s: s.index[0].start or 0)
        parts = list(ex.map(lambda s: np.asarray(s.data), shards))
        fetched.append(parts)
    DEBUG.setdefault("run_walls", []).append(_time.perf_counter() - t0)
    return [
        {nm: fetched[i][k] for i, nm in enumerate(out_names)}
        for k in range(NC_N)
    ]


def gnn_forward(x, edge_index, gate_w1, gate_b1, gate_w2, gate_b2,
                gcn_w1, gcn_b1, bn_gamma, bn_beta, gcn_w2, gcn_b2,
                gat_w1, gat_att_src1, gat_att_dst1, gat_b1,
                gat_w2, gat_att_src2, gat_att_dst2, gat_b2,
                sage_wl1, sage_bl1, sage_wr1, sage_wl2, sage_bl2, sage_wr2,
                prebuilt=None):
    n_nodes = x.shape[0]
    x = np.asarray(x, np.float32)
    streams, tiles_w, Tpad, shard, nw = build_schedule(
        np.asarray(edge_index), n_nodes)
    npad = nw * 128

    # ---- host weight folding (weights only, no data)
    w1r = np.asarray(gat_w1, np.float32).reshape(D, H1, D)
    vsrc = np.einsum("chj,hj->ch", w1r, np.asarray(gat_att_src1, np.float32))
    vdst = np.einsum("chj,hj->ch", w1r, np.asarray(gat_att_dst1, np.float32))
    vcat = np.concatenate([vsrc, vdst], axis=1).astype(np.float32)  # [64,8]
    v2 = (np.asarray(gat_w2, np.float32) @
          np.asarray(gat_att_src2, np.float32)[0])  # [256]
    u2 = (np.asarray(gat_w2, np.float32) @
          np.asarray(gat_att_dst2, np.float32)[0])
    v2u2 = np.stack([v2[:128], u2[:128], v2[128:], u2[128:]],
                    axis=1).astype(np.float32)  # [128,4]
    bn_s = (np.asarray(bn_gamma, np.float32) /
            np.sqrt(np.float32(1.0 + BN_EPS)))
    gcn1_s = bn_s.reshape(D, 1).astype(np.float32)
    gcn1_b = (bn_s * np.asarray(gcn_b1, np.float32) +
              np.asarray(bn_beta, np.float32)).reshape(D, 1).astype(np.float32)

    def pad_shard_f16(arr, k, width):
        out = np.zeros((npad, width), np.float16)
        out[:shard] = arr[k * shard:(k + 1) * shard]
        return out

    ck = (n_nodes, Tpad, tuple(tiles_w))
    if prebuilt is not None:
        nc_all = prebuilt
    elif ck in _PROG_CACHE:
        nc_all = _PROG_CACHE[ck]
    else:
        nc_all = build_all(n_nodes, shard, nw, tiles_w, Tpad)
        _PROG_CACHE[ck] = nc_all

    wvals = {
        "vcat": vcat,
        "gw1": np.asarray(gate_w1, np.float32),
        "gb1": np.asarray(gate_b1, np.float32).reshape(1, D),
        "gw2": np.asarray(gate_w2, np.float32),
        "gb2": np.asarray(gate_b2, np.float32).reshape(1, 3),
        "gcn_w1": np.asarray(gcn_w1, np.float32),
        "gcn1_s": gcn1_s, "gcn1_b": gcn1_b,
        "sage_wl1": np.asarray(sage_wl1, np.float32),
        "sage_wr1": np.asarray(sage_wr1, np.float32),
        "sage_bl1": np.asarray(sage_bl1, np.float32).reshape(D, 1),
        "w2A": np.asarray(gat_w2, np.float32)[:128],
        "w2B": np.asarray(gat_w2, np.float32)[128:],
        "v2u2": v2u2,
        "w1h": np.asarray(gat_w1, np.float32),
        "b1c": np.asarray(gat_b1, np.float32).reshape(2, 128).T.copy(),
        "gcn_w2": np.asarray(gcn_w2, np.float32),
        "gcn_b2c": np.asarray(gcn_b2, np.float32).reshape(D, 1),
        "sage_wl2": np.asarray(sage_wl2, np.float32),
        "sage_wr2": np.asarray(sage_wr2, np.float32),
        "sage_bl2c": np.asarray(sage_bl2, np.float32).reshape(D, 1),
        "gat_b2r": np.asarray(gat_b2, np.float32).reshape(1, D),
    }
    for nm, shp in WSPEC:
        assert wvals[nm].shape == shp, (nm, wvals[nm].shape, shp)
    wbpad = np.zeros(NC_N * WSH, np.float16)
    wbpad[:WTOT] = np.concatenate(
        [wvals[nm].ravel() for nm, _ in WSPEC]).astype(np.float16)

    in_maps = []
    for k in range(NC_N):
        fb = np.concatenate([
            pad_shard_f16(x, k, D).ravel(),
            streams[k]["dis16"],
            wbpad[k * WSH:(k + 1) * WSH],
        ]).reshape(1, -1)
        in_maps.append({"fb": fb, "iu16": streams[k]["iu16"]})
    res = _run(nc_all, in_maps)
    out = np.concatenate([res[k]["out"][:shard] for k in range(NC_N)], 0)
    return out.astype(np.float32)


def kernel(**inputs):
    return gnn_forward(**inputs)
